# revision 25
# baseline (speedup 1.0000x reference)
"""Trainium2 Bass kernel for nn_CAGKE_learnable_minmax.

Reference computation for X[1,8192], weight[1,128], sigma_min[1], sigma_max[1],
noise[1,8192]:
    sigmas_d = |smin + d*(smax-smin)/127|
    K[d,j]   = c/sigma_d * exp(-(j-T)^2/(2 sigma_d^2))   (16383-tap Gaussians)
    embed    = conv(mask(X), K)                           [128, 8192]
    psedu    = softmax(weight) @ embed + 0.01*noise
    out      = (psedu - min) / (max - min)

Algebraic collapse: softmax(w) @ (G conv m) == (softmax(w) @ G) conv m, and each
Gaussian underflows to exact fp32 zero beyond ~|58| taps, so the [128, 16383]
kernel bank collapses to ONE 128-tap effective kernel geff, evaluated on-device
from the runtime sigmas/weights via a PE matmul over the sigma axis. The 8192-
long conv is then 3 accumulated PE matmuls (contraction over within-block mask
position) whose Toeplitz moving operand is built by an overlapping-window DMA
from a 512-float DRAM scratch row. Global min/max: DVE row reduces + a GpSimd
partition_all_reduce (result broadcast to all partitions). Everything
(threshold, softmax, Gaussian evaluation, conv, noise add, min/max,
normalization) runs on device.

Host side does layout-only prep: the mask operand is passed pre-transposed /
block-reversed (XrevT = X.reshape(64,128)[:, ::-1].T) because PE matmul needs
the contraction axis on partitions and DMA requires a contiguous last dim, and
weight/sigma_min/sigma_max are concatenated into one row so a single descriptor
loads them. The tiny per-core work is replicated on all 8 cores (no
collectives); core 0's output is returned.
"""

import numpy as np

import concourse.bass as bass
import concourse.bacc as bacc
import concourse.mybir as mybir
import concourse.tile as tile
from concourse.bass_utils import run_bass_kernel_spmd

T = 8192
D = 128
NB = T // 128  # 64 blocks of 128 outputs
INV_SQRT_2PI = 0.39894228
NOISE_SIGMA = 0.01
F32 = mybir.dt.float32
I32 = mybir.dt.int32
AX = mybir.AxisListType
ALU = mybir.AluOpType
ACT = mybir.ActivationFunctionType


def _emit(tc, nc, h, swdge=False, f32r=False, par=False):
    sb_cm = tc.tile_pool(name="sb", bufs=1)
    pp_cm = tc.tile_pool(name="ps", bufs=1, space="PSUM")
    sb = sb_cm.__enter__()
    pp = pp_cm.__enter__()
    MMF = mybir.dt.float32r if f32r else F32

    # ---- constants (all off the critical path) ---------------------------
    ones = sb.tile([1, 128], F32, tag="ones")    # bcast matmul lhsT
    nc.gpsimd.memset(ones, 1.0)
    one1 = sb.tile([1, 1], F32, tag="one1")      # identity for [1,n] transpose
    nc.gpsimd.memset(one1, 1.0)
    ones128 = sb.tile([128, 1], F32, tag="ones128")  # softmax-Z matmul rhs
    nc.gpsimd.memset(ones128, 1.0)
    mpad = sb.tile([128, 66], MMF, tag="mpad")   # zero-padded maskT
    nc.gpsimd.memset(mpad.bitcast(F32), 0.0)
    gsb = sb.tile([1, 192], MMF, tag="gsb")      # zero flank row for gscr
    nc.vector.memset(gsb.bitcast(F32), 0.0)
    d_i32 = sb.tile([128, 1], I32, tag="d_i32")
    nc.gpsimd.iota(d_i32, pattern=[[0, 1]], base=0, channel_multiplier=1)
    u_i32 = sb.tile([128, 128], I32, tag="u_i32")  # x = col - 64
    nc.gpsimd.iota(u_i32, pattern=[[1, 128]], base=-64, channel_multiplier=0)
    dF = sb.tile([128, 1], F32, tag="dF")
    nc.vector.tensor_copy(out=dF, in_=d_i32)
    dF127 = sb.tile([128, 1], F32, tag="dF127")  # d/127
    nc.scalar.mul(out=dF127, in_=dF, mul=1.0 / (D - 1))
    uF = sb.tile([128, 128], F32, tag="uF")
    nc.vector.tensor_copy(out=uF, in_=u_i32)
    u2 = sb.tile([128, 128], F32, tag="u2")      # x^2
    nc.scalar.activation(out=u2, in_=uF, func=ACT.Square)

    # ---- input DMAs ------------------------------------------------------
    # wst row: [weight(128) | sigma_min | sigma_max | (stp computed later)]
    wst = sb.tile([1, 132], F32, tag="wst")
    nc.sync.dma_start(out=wst[:, 0:130], in_=bass.AP(h["wsig"], 0, [[130, 1], [1, 130]]))
    xrt = sb.tile([128, 64], F32, tag="xrt")
    nc.sync.dma_start(out=xrt, in_=bass.AP(h["XrevT"], 0, [[64, 128], [1, 64]]))
    nz = sb.tile([NB, 128], F32, tag="nz")
    nc.scalar.dma_start(out=nz, in_=bass.AP(h["noise"], 0, [[128, NB], [1, 128]]))

    # ---- mask: threshold (X>0.5) into the padded Toeplitz lhsT -----------
    nc.vector.tensor_scalar(
        out=mpad[:, 1:65], in0=xrt, scalar1=0.5, scalar2=None, op0=ALU.is_gt
    )

    # ---- softmax numerator: expw_d = exp(w_d); Z handled via 1/Z later ---
    wtp = pp.tile([128, 1], F32, tag="wtp")
    nc.tensor.transpose(wtp, wst[0:1, 0:128], one1)
    expw = sb.tile([128, 1], F32, tag="expw")
    nc.scalar.activation(out=expw, in_=wtp, func=ACT.Exp)
    zp = pp.tile([1, 1], F32, tag="zp")
    nc.tensor.matmul(zp, lhsT=expw, rhs=ones128, start=True, stop=True)
    rz = sb.tile([1, 1], F32, tag="rz")
    nc.vector.reciprocal(out=rz, in_=zp)

    # ---- sigmas: s_d = |smin + (d/127)*(smax-smin)| ----------------------
    nc.vector.tensor_sub(out=wst[:, 130:131], in0=wst[:, 129:130], in1=wst[:, 128:129])
    bp = pp.tile([128, 2], F32, tag="bp")  # broadcast (smin, stp) to all parts
    nc.tensor.matmul(bp, lhsT=ones, rhs=wst[0:1, 128:131:2], start=True, stop=True)
    sg = sb.tile([128, 1], F32, tag="sg")
    nc.vector.tensor_scalar(
        out=sg, in0=dF127, scalar1=bp[:, 1:2], scalar2=bp[:, 0:1],
        op0=ALU.mult, op1=ALU.add,
    )
    rsg = sb.tile([128, 1], F32, tag="rsg")      # 1/s_d (signed)
    nc.vector.reciprocal(out=rsg, in_=sg)
    amp = sb.tile([128, 1], F32, tag="amp")      # c/|s_d| = |c*rsg|
    nc.scalar.activation(out=amp, in_=rsg, func=ACT.Abs, scale=INV_SQRT_2PI)
    nh2 = sb.tile([128, 1], F32, tag="nh2")      # -1/(2 s_d^2)
    nc.vector.tensor_scalar(
        out=nh2, in0=rsg, scalar1=rsg, scalar2=-0.5, op0=ALU.mult, op1=ALU.mult,
    )

    # ---- Gaussian taps + sigma-axis collapse -----------------------------
    expt = sb.tile([128, 128], MMF, tag="expt")  # exp(-x^2/(2 s_d^2))
    nc.scalar.activation(out=expt, in_=u2, func=ACT.Exp, bias=0.0, scale=nh2)
    weff = sb.tile([128, 1], MMF, tag="weff")    # expw_d * c/s_d
    nc.vector.tensor_mul(out=weff, in0=expw, in1=amp)
    gp = pp.tile([1, 128], F32, tag="gp")        # geff(x), x in [-64, 64)
    nc.tensor.matmul(gp, lhsT=weff, rhs=expt, start=True, stop=True)
    # apply the 1/Z softmax scale while copying out of PSUM
    gper = sb.tile([1, 128], MMF, tag="gper")
    nc.vector.tensor_scalar_mul(out=gper, in0=gp, scalar1=rz)

    # ---- Toeplitz build via overlapping-window DMA roundtrip -------------
    # zero flanks [0:192) and [320:512) are input-independent: written early so
    # the rt reads only wait on the tiny 512B geff write (single descriptor).
    rdma = nc.gpsimd if swdge else nc.sync
    rdma2 = nc.gpsimd if swdge else nc.scalar
    rdma.dma_start(out=bass.AP(h["gscr"], 0, [[1, 192]]), in_=gsb[:, 0:192])
    rdma2.dma_start(out=bass.AP(h["gscr"], 320, [[1, 192]]), in_=gsb[:, 0:192])
    rdma.dma_start(out=bass.AP(h["gscr"], 192, [[1, 128]]), in_=gper)
    # three chunks on alternating queues so the first conv can start earliest
    rta = sb.tile([128, 128], MMF, tag="rta")    # rt[k,v] = gscr[k+v]
    rdma.dma_start(out=rta, in_=bass.AP(h["gscr"], 0, [[1, 128], [1, 128]]))
    rtb = sb.tile([128, 128], MMF, tag="rtb")
    rdma2.dma_start(out=rtb, in_=bass.AP(h["gscr"], 128, [[1, 128], [1, 128]]))
    rtc = sb.tile([128, 128], MMF, tag="rtc")
    rdma.dma_start(out=rtc, in_=bass.AP(h["gscr"], 256, [[1, 128], [1, 128]]))

    # ---- conv: psedu[128b+j] = sum_{k,delta} mask*geff -------------------
    cp = pp.tile([NB, 128], F32, tag="cp")
    nc.tensor.matmul(cp, lhsT=mpad[:, 2:66], rhs=rta, start=True, stop=False)
    nc.tensor.matmul(cp, lhsT=mpad[:, 1:65], rhs=rtb, start=False, stop=False)
    nc.tensor.matmul(cp, lhsT=mpad[:, 0:64], rhs=rtc, start=False, stop=True)

    # ---- + noise; global min/max; normalize ------------------------------
    nz01 = sb.tile([NB, 128], F32, tag="nz01")
    nc.gpsimd.tensor_scalar_mul(out=nz01, in0=nz, scalar1=NOISE_SIGMA)
    ps = sb.tile([NB, 128], F32, tag="ps")
    mm = sb.tile([NB, 2], F32, tag="mm")
    nc.vector.tensor_add(out=ps, in0=cp, in1=nz01)
    nc.vector.reduce_max(out=mm[:, 0:1], in_=ps, axis=AX.X)
    nc.vector.tensor_reduce(out=mm[:, 1:2], in_=ps, axis=AX.X, op=ALU.min, negate=True)
    outx = sb.tile([NB, 128], F32, tag="outx")
    if par:
        from concourse import bass_isa
        pr = sb.tile([NB, 2], F32, tag="pr")  # every partition gets (hi, -lo)
        nc.gpsimd.partition_all_reduce(pr, mm, channels=NB,
                                       reduce_op=bass_isa.ReduceOp.max)
        rng = sb.tile([NB, 1], F32, tag="rng")
        nc.vector.tensor_add(out=rng, in0=pr[:, 0:1], in1=pr[:, 1:2])
        inv = sb.tile([NB, 1], F32, tag="inv")
        nc.vector.reciprocal(out=inv, in_=rng)
        nc.vector.tensor_scalar(
            out=outx, in0=ps, scalar1=pr[:, 1:2], scalar2=inv,
            op0=ALU.add, op1=ALU.mult,
        )
    else:
        sc = sb.tile([1, 4], F32, tag="sc")
        nc.gpsimd.tensor_reduce(out=sc[:, 0:2], in_=mm, axis=AX.C, op=ALU.max)
        nc.vector.tensor_add(out=sc[:, 2:3], in0=sc[:, 0:1], in1=sc[:, 1:2])
        nc.vector.reciprocal(out=sc[:, 3:4], in_=sc[:, 2:3])
        bc = pp.tile([NB, 2], F32, tag="bc")  # broadcast (-lo, inv) to 64 parts
        nc.tensor.matmul(bc, lhsT=ones[:, 0:NB], rhs=sc[0:1, 1:4:2],
                         start=True, stop=True)
        nc.vector.tensor_scalar(
            out=outx, in0=ps, scalar1=bc[:, 0:1], scalar2=bc[:, 1:2],
            op0=ALU.add, op1=ALU.mult,
        )
    nc.scalar.dma_start(out=bass.AP(h["out"], 0, [[128, NB], [1, 128]]), in_=outx)

    sb_cm.__exit__(None, None, None)
    pp_cm.__exit__(None, None, None)


def build_nc(swdge=False, f32r=False, par=False):
    nc = bacc.Bacc("TRN2", debug=False, enable_partition_id=False)
    h = {
        "XrevT": nc.dram_tensor("XrevT", [128, NB], F32, kind="ExternalInput"),
        "wsig": nc.dram_tensor("wsig", [1, 130], F32, kind="ExternalInput"),
        "noise": nc.dram_tensor("noise", [1, T], F32, kind="ExternalInput"),
        "out": nc.dram_tensor("out", [1, T], F32, kind="ExternalOutput"),
        "gscr": nc.dram_tensor("gscr", [512],
                               mybir.dt.float32r if f32r else F32, kind="Internal"),
    }
    with tile.TileContext(nc) as tc:
        _emit(tc, nc, h, swdge=swdge, f32r=f32r, par=par)
    nc.compile()
    return nc


_NC_CACHE = None


def _get_nc():
    global _NC_CACHE
    if _NC_CACHE is None:
        _NC_CACHE = build_nc(**_CONFIG)
    return _NC_CACHE


_CONFIG = {"swdge": False, "f32r": False, "par": True}


def _prep_inputs(inputs):
    """Layout-only host prep (reshape/transpose/flip/concat -- no arithmetic)."""
    X = np.asarray(inputs["X"], dtype=np.float32)
    weight = np.asarray(inputs["weight"], dtype=np.float32)
    smin = np.asarray(inputs["sigma_min"], dtype=np.float32)
    smax = np.asarray(inputs["sigma_max"], dtype=np.float32)
    noise = np.asarray(inputs["noise"], dtype=np.float32)
    xrevt = np.ascontiguousarray(X.reshape(NB, 128)[:, ::-1].T)
    wsig = np.ascontiguousarray(
        np.concatenate(
            [weight.reshape(1, D), smin.reshape(1, 1), smax.reshape(1, 1)], axis=1
        )
    )
    return {
        "XrevT": xrevt,
        "wsig": wsig,
        "noise": np.ascontiguousarray(noise.reshape(1, T)),
    }


def kernel(**inputs: np.ndarray) -> np.ndarray:
    nc = _get_nc()
    in_map = _prep_inputs(inputs)
    n_cores = 8
    res = run_bass_kernel_spmd(nc, [in_map] * n_cores, core_ids=list(range(n_cores)))
    return res.results[0]["out"].reshape(1, T)


# revision 29
# speedup vs baseline: 1.0084x; 1.0084x over previous
"""Trainium2 Bass kernel for nn_CAGKE_learnable_minmax.

Reference computation for X[1,8192], weight[1,128], sigma_min[1], sigma_max[1],
noise[1,8192]:
    sigmas_d = |smin + d*(smax-smin)/127|
    K[d,j]   = c/sigma_d * exp(-(j-T)^2/(2 sigma_d^2))   (16383-tap Gaussians)
    embed    = conv(mask(X), K)                           [128, 8192]
    psedu    = softmax(weight) @ embed + 0.01*noise
    out      = (psedu - min) / (max - min)

Algebraic collapse: softmax(w) @ (G conv m) == (softmax(w) @ G) conv m, and each
Gaussian underflows to exact fp32 zero beyond ~|58| taps, so the [128, 16383]
kernel bank collapses to ONE 128-tap effective kernel geff, evaluated on-device
from the runtime sigmas/weights via a PE matmul over the sigma axis. The 8192-
long conv is then 2 accumulated PE matmuls over half-block-shifted K=128 mask
windows (each 128-output block has a 242-wide receptive field, which two
shifted windows cover exactly) whose Toeplitz moving operand is built by an
overlapping-window DMA from a 384-float DRAM scratch row. Global min/max: DVE row reduces + a GpSimd
partition_all_reduce (result broadcast to all partitions). Everything
(threshold, softmax, Gaussian evaluation, conv, noise add, min/max,
normalization) runs on device.

Host side does layout-only prep: the two mask operands are passed
pre-transposed / block-reversed / half-block-shifted because PE matmul needs
the contraction axis on partitions and DMA requires a contiguous last dim, and
weight/sigma_min/sigma_max are concatenated into one row so a single descriptor
loads them. The tiny per-core work is replicated on all 8 cores (no
collectives); core 0's output is returned.
"""

import numpy as np

import concourse.bass as bass
import concourse.bacc as bacc
import concourse.mybir as mybir
import concourse.tile as tile
from concourse.bass_utils import run_bass_kernel_spmd

T = 8192
D = 128
NB = T // 128  # 64 blocks of 128 outputs
INV_SQRT_2PI = 0.39894228
NOISE_SIGMA = 0.01
F32 = mybir.dt.float32
I32 = mybir.dt.int32
AX = mybir.AxisListType
ALU = mybir.AluOpType
ACT = mybir.ActivationFunctionType


def _emit(tc, nc, h, swdge=False, f32r=False, par=False):
    sb_cm = tc.tile_pool(name="sb", bufs=1)
    pp_cm = tc.tile_pool(name="ps", bufs=1, space="PSUM")
    sb = sb_cm.__enter__()
    pp = pp_cm.__enter__()
    MMF = mybir.dt.float32r if f32r else F32

    # ---- constants (all off the critical path) ---------------------------
    ones = sb.tile([1, 128], F32, tag="ones")    # bcast matmul lhsT
    nc.gpsimd.memset(ones, 1.0)
    one1 = sb.tile([1, 1], F32, tag="one1")      # identity for [1,n] transpose
    nc.gpsimd.memset(one1, 1.0)
    ones128 = sb.tile([128, 1], F32, tag="ones128")  # softmax-Z matmul rhs
    nc.gpsimd.memset(ones128, 1.0)
    gsb = sb.tile([1, 128], MMF, tag="gsb")      # zero flank row for gscr
    nc.vector.memset(gsb.bitcast(F32), 0.0)
    d_i32 = sb.tile([128, 1], I32, tag="d_i32")
    nc.gpsimd.iota(d_i32, pattern=[[0, 1]], base=0, channel_multiplier=1)
    u_i32 = sb.tile([128, 128], I32, tag="u_i32")  # x = col - 64
    nc.gpsimd.iota(u_i32, pattern=[[1, 128]], base=-64, channel_multiplier=0)
    dF = sb.tile([128, 1], F32, tag="dF")
    nc.vector.tensor_copy(out=dF, in_=d_i32)
    dF127 = sb.tile([128, 1], F32, tag="dF127")  # d/127
    nc.scalar.mul(out=dF127, in_=dF, mul=1.0 / (D - 1))
    uF = sb.tile([128, 128], F32, tag="uF")
    nc.vector.tensor_copy(out=uF, in_=u_i32)
    u2 = sb.tile([128, 128], F32, tag="u2")      # x^2
    nc.scalar.activation(out=u2, in_=uF, func=ACT.Square)

    # ---- input DMAs ------------------------------------------------------
    # wst row: [weight(128) | sigma_min | sigma_max | (stp computed later)]
    wst = sb.tile([1, 132], F32, tag="wst")
    nc.sync.dma_start(out=wst[:, 0:130], in_=bass.AP(h["wsig"], 0, [[130, 1], [1, 130]]))
    xra = sb.tile([128, 64], F32, tag="xra")
    nc.sync.dma_start(out=xra, in_=bass.AP(h["XA"], 0, [[64, 128], [1, 64]]))
    xrb = sb.tile([128, 64], F32, tag="xrb")
    nc.scalar.dma_start(out=xrb, in_=bass.AP(h["XB"], 0, [[64, 128], [1, 64]]))
    nz = sb.tile([NB, 128], F32, tag="nz")
    nc.scalar.dma_start(out=nz, in_=bass.AP(h["noise"], 0, [[128, NB], [1, 128]]))

    # ---- softmax numerator: expw_d = exp(w_d); Z handled via 1/Z later ---
    wtp = pp.tile([128, 1], F32, tag="wtp")
    nc.tensor.transpose(wtp, wst[0:1, 0:128], one1)
    expw = sb.tile([128, 1], F32, tag="expw")
    nc.scalar.activation(out=expw, in_=wtp, func=ACT.Exp)
    zp = pp.tile([1, 1], F32, tag="zp")
    nc.tensor.matmul(zp, lhsT=expw, rhs=ones128, start=True, stop=True)
    rz = sb.tile([1, 1], F32, tag="rz")
    nc.vector.reciprocal(out=rz, in_=zp)

    # ---- sigmas: s_d = |smin + (d/127)*(smax-smin)| ----------------------
    nc.vector.tensor_sub(out=wst[:, 130:131], in0=wst[:, 129:130], in1=wst[:, 128:129])
    bp = pp.tile([128, 2], F32, tag="bp")  # broadcast (smin, stp) to all parts
    nc.tensor.matmul(bp, lhsT=ones, rhs=wst[0:1, 128:131:2], start=True, stop=True)
    sg = sb.tile([128, 1], F32, tag="sg")
    nc.vector.tensor_scalar(
        out=sg, in0=dF127, scalar1=bp[:, 1:2], scalar2=bp[:, 0:1],
        op0=ALU.mult, op1=ALU.add,
    )
    rsg = sb.tile([128, 1], F32, tag="rsg")      # 1/s_d (signed)
    nc.vector.reciprocal(out=rsg, in_=sg)
    amp = sb.tile([128, 1], F32, tag="amp")      # c/|s_d| = |c*rsg|
    nc.scalar.activation(out=amp, in_=rsg, func=ACT.Abs, scale=INV_SQRT_2PI)
    nh2 = sb.tile([128, 1], F32, tag="nh2")      # -1/(2 s_d^2)
    nc.vector.tensor_scalar(
        out=nh2, in0=rsg, scalar1=rsg, scalar2=-0.5, op0=ALU.mult, op1=ALU.mult,
    )

    # ---- Gaussian taps + sigma-axis collapse -----------------------------
    expt = sb.tile([128, 128], MMF, tag="expt")  # exp(-x^2/(2 s_d^2))
    nc.scalar.activation(out=expt, in_=u2, func=ACT.Exp, bias=0.0, scale=nh2)
    weff = sb.tile([128, 1], MMF, tag="weff")    # expw_d * c/s_d
    nc.vector.tensor_mul(out=weff, in0=expw, in1=amp)
    gp = pp.tile([1, 128], F32, tag="gp")        # geff(x), x in [-64, 64)
    nc.tensor.matmul(gp, lhsT=weff, rhs=expt, start=True, stop=True)
    # apply the 1/Z softmax scale while copying out of PSUM
    gper = sb.tile([1, 128], MMF, tag="gper")
    nc.vector.tensor_scalar_mul(out=gper, in0=gp, scalar1=rz)

    # ---- Toeplitz build via overlapping-window DMA roundtrip -------------
    # gscr2[u] = geff(u-192), u in [0,384). Zero flanks [0:128) and [256:384)
    # are input-independent and written early, so the rt reads only wait on
    # the single-descriptor 512B geff write.
    rdma = nc.gpsimd if swdge else nc.sync
    rdma2 = nc.gpsimd if swdge else nc.scalar
    rdma.dma_start(out=bass.AP(h["gscr"], 0, [[1, 128]]), in_=gsb)
    rdma2.dma_start(out=bass.AP(h["gscr"], 256, [[1, 128]]), in_=gsb)
    rdma.dma_start(out=bass.AP(h["gscr"], 128, [[1, 128]]), in_=gper)
    rt0 = sb.tile([128, 128], MMF, tag="rt0")    # rt[k,v] = gscr2[k+v]
    rdma.dma_start(out=rt0, in_=bass.AP(h["gscr"], 0, [[1, 128], [1, 128]]))
    rt1 = sb.tile([128, 128], MMF, tag="rt1")    # rt[k,v] = gscr2[k+v+128]
    rdma2.dma_start(out=rt1, in_=bass.AP(h["gscr"], 128, [[1, 128], [1, 128]]))

    # ---- mask: threshold (X>0.5); mA[k,b]=m[128b+63-k], mB[k,b]=m[128b+191-k]
    mA = sb.tile([128, 64], MMF, tag="mA")
    nc.gpsimd.tensor_scalar(out=mA, in0=xra, scalar1=0.5, scalar2=None, op0=ALU.is_gt)
    mB = sb.tile([128, 64], MMF, tag="mB")
    nc.gpsimd.tensor_scalar(out=mB, in0=xrb, scalar1=0.5, scalar2=None, op0=ALU.is_gt)

    # ---- conv: psedu[128b+j] = sum over two half-shifted K=128 windows ---
    cp = pp.tile([NB, 128], F32, tag="cp")
    nc.tensor.matmul(cp, lhsT=mA, rhs=rt1, start=True, stop=False)
    nc.tensor.matmul(cp, lhsT=mB, rhs=rt0, start=False, stop=True)

    # ---- + noise; global min/max; normalize ------------------------------
    nz01 = sb.tile([NB, 128], F32, tag="nz01")
    nc.scalar.mul(out=nz01, in_=nz, mul=NOISE_SIGMA)
    ps = sb.tile([NB, 128], F32, tag="ps")
    mm = sb.tile([NB, 2], F32, tag="mm")
    nc.vector.tensor_add(out=ps, in0=cp, in1=nz01)
    nc.vector.reduce_max(out=mm[:, 0:1], in_=ps, axis=AX.X)
    nc.vector.tensor_reduce(out=mm[:, 1:2], in_=ps, axis=AX.X, op=ALU.min, negate=True)
    outx = sb.tile([NB, 128], F32, tag="outx")
    if par:
        from concourse import bass_isa
        pr = sb.tile([NB, 2], F32, tag="pr")  # every partition gets (hi, -lo)
        nc.gpsimd.partition_all_reduce(pr, mm, channels=NB,
                                       reduce_op=bass_isa.ReduceOp.max)
        rng = sb.tile([NB, 1], F32, tag="rng")
        nc.vector.tensor_add(out=rng, in0=pr[:, 0:1], in1=pr[:, 1:2])
        inv = sb.tile([NB, 1], F32, tag="inv")
        nc.vector.reciprocal(out=inv, in_=rng)
        nc.vector.tensor_scalar(
            out=outx, in0=ps, scalar1=pr[:, 1:2], scalar2=inv,
            op0=ALU.add, op1=ALU.mult,
        )
    else:
        sc = sb.tile([1, 4], F32, tag="sc")
        nc.gpsimd.tensor_reduce(out=sc[:, 0:2], in_=mm, axis=AX.C, op=ALU.max)
        nc.vector.tensor_add(out=sc[:, 2:3], in0=sc[:, 0:1], in1=sc[:, 1:2])
        nc.vector.reciprocal(out=sc[:, 3:4], in_=sc[:, 2:3])
        bc = pp.tile([NB, 2], F32, tag="bc")  # broadcast (-lo, inv) to 64 parts
        nc.tensor.matmul(bc, lhsT=ones[:, 0:NB], rhs=sc[0:1, 1:4:2],
                         start=True, stop=True)
        nc.vector.tensor_scalar(
            out=outx, in0=ps, scalar1=bc[:, 0:1], scalar2=bc[:, 1:2],
            op0=ALU.add, op1=ALU.mult,
        )
    nc.scalar.dma_start(out=bass.AP(h["out"], 0, [[128, NB], [1, 128]]), in_=outx)

    sb_cm.__exit__(None, None, None)
    pp_cm.__exit__(None, None, None)


def build_nc(swdge=False, f32r=False, par=False):
    nc = bacc.Bacc("TRN2", debug=False, enable_partition_id=False)
    h = {
        "XA": nc.dram_tensor("XA", [128, NB], F32, kind="ExternalInput"),
        "XB": nc.dram_tensor("XB", [128, NB], F32, kind="ExternalInput"),
        "wsig": nc.dram_tensor("wsig", [1, 130], F32, kind="ExternalInput"),
        "noise": nc.dram_tensor("noise", [1, T], F32, kind="ExternalInput"),
        "out": nc.dram_tensor("out", [1, T], F32, kind="ExternalOutput"),
        "gscr": nc.dram_tensor("gscr", [384],
                               mybir.dt.float32r if f32r else F32, kind="Internal"),
    }
    with tile.TileContext(nc) as tc:
        _emit(tc, nc, h, swdge=swdge, f32r=f32r, par=par)
    nc.compile()
    return nc


_NC_CACHE = None


def _get_nc():
    global _NC_CACHE
    if _NC_CACHE is None:
        _NC_CACHE = build_nc(**_CONFIG)
    return _NC_CACHE


_CONFIG = {"swdge": False, "f32r": False, "par": True}


def _prep_inputs(inputs):
    """Layout-only host prep (reshape/transpose/flip/concat -- no arithmetic)."""
    X = np.asarray(inputs["X"], dtype=np.float32)
    weight = np.asarray(inputs["weight"], dtype=np.float32)
    smin = np.asarray(inputs["sigma_min"], dtype=np.float32)
    smax = np.asarray(inputs["sigma_max"], dtype=np.float32)
    noise = np.asarray(inputs["noise"], dtype=np.float32)
    xf = X.reshape(T)
    xpa = np.concatenate([np.zeros(64, np.float32), xf])[:T]
    xpb = np.concatenate([xf[64:], np.zeros(64, np.float32)])
    xra = np.ascontiguousarray(xpa.reshape(NB, 128)[:, ::-1].T)  # m[128b+63-k]
    xrb = np.ascontiguousarray(xpb.reshape(NB, 128)[:, ::-1].T)  # m[128b+191-k]
    wsig = np.ascontiguousarray(
        np.concatenate(
            [weight.reshape(1, D), smin.reshape(1, 1), smax.reshape(1, 1)], axis=1
        )
    )
    return {
        "XA": xra,
        "XB": xrb,
        "wsig": wsig,
        "noise": np.ascontiguousarray(noise.reshape(1, T)),
    }


def kernel(**inputs: np.ndarray) -> np.ndarray:
    nc = _get_nc()
    in_map = _prep_inputs(inputs)
    n_cores = 8
    res = run_bass_kernel_spmd(nc, [in_map] * n_cores, core_ids=list(range(n_cores)))
    return res.results[0]["out"].reshape(1, T)
